# revision 7
# baseline (speedup 1.0000x reference)
"""Trainium2 8-core tensor-parallel attention kernel (Bass/Tile).

Problem: B=1, S=2048, D=4096, H=32 q-heads, KVH=8 kv-heads, HD=128, causal,
RoPE, GQA.  Sharding: tensor-parallel over heads — each of the 8 cores owns
4 q heads + 1 kv head (wq/wk/wv column shards), computes its heads' attention
output transposed [512, S], AllGathers those into [4096, S], and then computes
a 512-wide column shard of the final wo projection.  Host assembles the full
[1, S, 4096] output by concatenating the per-core column shards.

Compute dtype: bf16 operands / f32 accumulation (rel err ~0.8% vs f32 ref).
Layouts are host-prepared so every DMA is a contiguous partition-major image:
  xtr  [4, 128, 32*512]  x^T tiles: xtr[T][p, 512*b+f] = x[512*T+f, 128*b+p]
  wqkv [128, 32*768]     [wq_perm | wk_perm | wv] d-block-major stationary tiles
  woc  [128, 32*512]     wo column shard, d-block-major
  cst/snt [64, 2048]     RoPE tables transposed (f32)
  m01  [128, 4*512]      0/1 causal masks for the 4 diagonal block offsets
RoPE trick: wq/wk columns are permuted per head (even channels first) so the
rotation acts on partition halves [0:64]/[64:128] of the projected qT/kT.
Softmax is computed in the transposed score layout (scoresT[sk, sq]) without
max-subtraction (|scores| <~ 15 so exp stays in f32/bf16 range); column sums
via a ones-vector matmul, normalization via a rank-1 outer-product matmul.
"""

import math

import numpy as np
import ml_dtypes

import concourse.bass as bass
import concourse.bacc as bacc
import concourse.tile as tile
import concourse.mybir as mybir
from concourse.bass_utils import run_bass_kernel_spmd

BF16 = mybir.dt.bfloat16
F32 = mybir.dt.float32
NPBF = ml_dtypes.bfloat16

N_CORES = 8
S, D, H, KVH, HD = 2048, 4096, 32, 8, 128
QH = H // N_CORES          # 4 q heads per core
NB = D // 128              # 32 contraction blocks
ST = S // 512              # 4 s-tiles
SCALE = 1.0 / math.sqrt(HD)

_CACHE = {}


def build_kernel(n_cores=N_CORES, with_collective=True, reps=1):
    nc = bacc.Bacc("TRN2", target_bir_lowering=False, debug=False,
                   num_devices=n_cores)

    xtr = nc.declare_dram_parameter("xtr", [ST, 128, NB * 512], BF16, isOutput=False)
    wqkv = nc.declare_dram_parameter("wqkv", [128, NB * 768], BF16, isOutput=False)
    woc = nc.declare_dram_parameter("woc", [128, NB * 512], BF16, isOutput=False)
    cst = nc.declare_dram_parameter("cst", [64, S], F32, isOutput=False)
    snt = nc.declare_dram_parameter("snt", [64, S], F32, isOutput=False)
    m01 = nc.declare_dram_parameter("m01", [128, 4 * 512], BF16, isOutput=False)
    ident = nc.declare_dram_parameter("ident", [128, 128], BF16, isOutput=False)
    out = nc.declare_dram_parameter("out", [S, 512], F32, isOutput=True)

    with tile.TileContext(nc) as tc:
        with (
            tc.tile_pool(name="const", bufs=1) as constp,
            tc.tile_pool(name="dram", bufs=1, space="DRAM") as dram,
        ):
            # persistent SBUF images
            wqkv_sb = constp.tile([128, NB * 768], BF16)
            nc.sync.dma_start(wqkv_sb[:], wqkv[:])
            cst_sb = constp.tile([64, S], F32)
            nc.sync.dma_start(cst_sb[:], cst[:])
            snt_sb = constp.tile([64, S], F32)
            nc.sync.dma_start(snt_sb[:], snt[:])
            m01_sb = constp.tile([128, 4 * 512], BF16)
            nc.sync.dma_start(m01_sb[:], m01[:])
            ident_sb = constp.tile([128, 128], BF16)
            nc.sync.dma_start(ident_sb[:], ident[:])
            ones_sb = constp.tile([128, 128], BF16)
            nc.gpsimd.memset(ones_sb[:], 1.0)

            cc_in = dram.tile([QH * HD, S], BF16)
            cc_out = dram.tile([N_CORES * QH * HD, S], BF16, addr_space="Shared")

            for rep in range(reps):
                _emit_body(nc, tc, n_cores, with_collective,
                           xtr, woc, out, wqkv_sb, cst_sb, snt_sb,
                           m01_sb, ident_sb, ones_sb, cc_in, cc_out)

    nc.compile()
    return nc


def _emit_body(nc, tc, n_cores, with_collective, xtr, woc, out, wqkv_sb,
               cst_sb, snt_sb, m01_sb, ident_sb, ones_sb, cc_in, cc_out):
    with (
        tc.tile_pool(name="persist", bufs=1) as pers,
        tc.tile_pool(name="xs", bufs=2) as xpool,
        tc.tile_pool(name="work", bufs=2) as work,
        tc.tile_pool(name="exps", bufs=4) as epool,
        tc.tile_pool(name="psA", bufs=2, space="PSUM") as psA,
        tc.tile_pool(name="psB", bufs=2, space="PSUM") as psB,
        tc.tile_pool(name="psC", bufs=2, space="PSUM") as psC,
        tc.tile_pool(name="psD", bufs=1, space="PSUM") as psD,
    ):
        # per-core activations (bf16)
        q_sb = [pers.tile([128, S], BF16, name=f"q{h}_sb") for h in range(QH)]
        kt_sb = pers.tile([128, S], BF16)
        v_sb = pers.tile([128, S], BF16)          # [sk within blk, hd] per 128-col block

        for T in range(ST):
            xt_sb = xpool.tile([128, NB * 512], BF16, tag="xt")
            nc.sync.dma_start(xt_sb[:], xtr[T])

            scol = slice(512 * T, 512 * T + 512)
            # ---- projections for this s-tile: 4 q heads, k, v ----
            for n in range(6):
                pp = psA.tile([128, 512], F32, tag="proj")
                for d in range(NB):
                    nc.tensor.matmul(
                        pp[:],
                        lhsT=wqkv_sb[:, 768 * d + 128 * n: 768 * d + 128 * n + 128],
                        rhs=xt_sb[:, 512 * d: 512 * d + 512],
                        start=(d == 0), stop=(d == NB - 1),
                    )
                if n < 5:
                    # RoPE: rows 0:64 real, 64:128 imag (weights were permuted)
                    dst = q_sb[n] if n < QH else kt_sb
                    c = cst_sb[:, scol]
                    s_ = snt_sb[:, scol]
                    t1 = work.tile([64, 512], F32, tag="rt1")
                    t2 = work.tile([64, 512], F32, tag="rt2")
                    nc.vector.tensor_mul(t1[:], pp[0:64, :], c)
                    nc.vector.tensor_mul(t2[:], pp[64:128, :], s_)
                    nc.vector.tensor_sub(dst[0:64, scol], t1[:], t2[:])
                    t3 = work.tile([64, 512], F32, tag="rt1")
                    t4 = work.tile([64, 512], F32, tag="rt2")
                    nc.vector.tensor_mul(t3[:], pp[0:64, :], s_)
                    nc.vector.tensor_mul(t4[:], pp[64:128, :], c)
                    nc.vector.tensor_add(dst[64:128, scol], t3[:], t4[:])
                else:
                    # V: cast to bf16, then PE-transpose 128x128 blocks into [sk, hd]
                    vt = work.tile([128, 512], BF16, tag="vt")
                    nc.scalar.copy(vt[:], pp[:])
                    for j in range(4):
                        pst = psD.tile([128, 128], BF16, tag="misc")
                        nc.tensor.transpose(pst[:], vt[:, 128 * j: 128 * j + 128],
                                            ident_sb[:])
                        nc.vector.tensor_copy(
                            v_sb[:, 128 * (4 * T + j): 128 * (4 * T + j) + 128],
                            pst[:])

            # ---- attention for this s-tile, all 4 heads ----
            nsk = 4 * (T + 1)
            for h in range(QH):
                op = psB.tile([128, 512], F32, tag="outp")
                sm = psD.tile([1, 512], F32, tag="sums")
                for b in range(nsk):
                    sc = psC.tile([128, 512], F32, tag="sc")
                    nc.tensor.matmul(
                        sc[:],
                        lhsT=kt_sb[:, 128 * b: 128 * b + 128],
                        rhs=q_sb[h][:, scol],
                        start=True, stop=True,
                    )
                    e = epool.tile([128, 512], BF16, tag="e")
                    nc.scalar.activation(e[:], sc[:],
                                         mybir.ActivationFunctionType.Exp,
                                         scale=SCALE)
                    r = b - 4 * T
                    if r >= 0:
                        nc.vector.tensor_mul(
                            e[:], e[:], m01_sb[:, 512 * r: 512 * r + 512])
                    nc.tensor.matmul(sm[0:1, :], lhsT=ones_sb[:, 0:1], rhs=e[:],
                                     start=(b == 0), stop=(b == nsk - 1))
                    nc.tensor.matmul(op[:],
                                     lhsT=v_sb[:, 128 * b: 128 * b + 128],
                                     rhs=e[:],
                                     start=(b == 0), stop=(b == nsk - 1))
                # normalize: og = op * (1/sums) broadcast across partitions
                r32 = work.tile([1, 512], F32, tag="r32")
                nc.vector.reciprocal(r32[:], sm[0:1, :])
                rbf = work.tile([1, 512], BF16, tag="rbf")
                nc.vector.tensor_copy(rbf[:], r32[:])
                bc = psD.tile([128, 512], F32, tag="misc")
                nc.tensor.matmul(bc[:], lhsT=ones_sb[0:1, 0:128], rhs=rbf[:],
                                 start=True, stop=True)
                bcs = work.tile([128, 512], BF16, tag="bcs")
                nc.scalar.copy(bcs[:], bc[:])
                og = work.tile([128, 512], BF16, tag="og", bufs=3)
                nc.vector.tensor_mul(og[:], op[:], bcs[:])
                nc.sync.dma_start(cc_in[128 * h: 128 * h + 128, scol], og[:])

    # ---- AllGather attention outputs across cores ----
    if with_collective:
        nc.gpsimd.collective_compute(
            "AllGather",
            mybir.AluOpType.bypass,
            replica_groups=[list(range(n_cores))],
            ins=[cc_in.opt()],
            outs=[cc_out.opt()],
        )
    else:
        # timing-only stand-in: fill own shard slot locally
        nc.sync.dma_start(cc_out[0: QH * HD, :], cc_in[:])

    # ---- final projection: out[:, core cols] = outT_all^T @ woc ----
    with (
        tc.tile_pool(name="fing", bufs=2) as gpool,
        tc.tile_pool(name="finw", bufs=1) as wpool,
        tc.tile_pool(name="fino", bufs=3) as opool,
        tc.tile_pool(name="psF", bufs=4, space="PSUM") as psF,
    ):
        woc_sb = wpool.tile([128, NB * 512], BF16)
        nc.sync.dma_start(woc_sb[:], woc[:])
        cc_view = cc_out[:, :].rearrange("(b p) s -> p b s", p=128)
        for qt in range(4):
            gsb = gpool.tile([128, NB, 512], BF16, tag="g")
            nc.sync.dma_start(gsb[:], cc_view[:, :, 512 * qt: 512 * qt + 512])
            for j in range(4):
                pf = psF.tile([128, 512], F32, tag="fin")
                for b in range(NB):
                    nc.tensor.matmul(
                        pf[:],
                        lhsT=gsb[:, b, 128 * j: 128 * j + 128],
                        rhs=woc_sb[:, 512 * b: 512 * b + 512],
                        start=(b == 0), stop=(b == NB - 1),
                    )
                osb = opool.tile([128, 512], F32, tag="o")
                nc.vector.tensor_copy(osb[:], pf[:])
                nc.sync.dma_start(out[512 * qt + 128 * j: 512 * qt + 128 * j + 128, :],
                                  osb[:])


# ---------------------------------------------------------------------------
# host-side preparation


def _prep_in_maps(x, wq, wk, wv, wo, freqs_cos, freqs_sin, mask):
    x2 = np.asarray(x, np.float32).reshape(S, D)
    perm = np.concatenate([np.arange(0, HD, 2), np.arange(1, HD, 2)])

    xT = np.ascontiguousarray(x2.T).astype(NPBF)                 # [D, S]
    # xtr[T][p, 512*b+f] = xT[128*b+p, 512*T+f]
    xtr = (xT.reshape(NB, 128, ST, 512).transpose(2, 1, 0, 3)
           .reshape(ST, 128, NB * 512))
    xtr = np.ascontiguousarray(xtr)

    cstn = np.ascontiguousarray(np.asarray(freqs_cos, np.float32).T)  # [64, S]
    sntn = np.ascontiguousarray(np.asarray(freqs_sin, np.float32).T)

    mnp = np.asarray(mask, np.float32)
    m01 = np.zeros((128, 4 * 512), np.float32)
    for r in range(4):
        m01[:, 512 * r: 512 * r + 512] = (mnp[0:512, 128 * r: 128 * r + 128].T == 0.0)
    m01 = m01.astype(NPBF)

    identity = np.eye(128, dtype=NPBF)

    wqn = np.asarray(wq, np.float32)
    wkn = np.asarray(wk, np.float32)
    wvn = np.asarray(wv, np.float32)
    won = np.asarray(wo, np.float32)

    in_maps = []
    for c in range(N_CORES):
        wq_c = wqn[:, c * QH * HD:(c + 1) * QH * HD].reshape(D, QH, HD)[:, :, perm]
        wq_c = wq_c.reshape(D, QH * HD)
        wk_c = wkn[:, c * HD:(c + 1) * HD][:, perm]
        wv_c = wvn[:, c * HD:(c + 1) * HD]
        wqkv_c = np.concatenate([wq_c, wk_c, wv_c], axis=1)      # [D, 768]
        wqkv_img = (wqkv_c.reshape(NB, 128, 768).transpose(1, 0, 2)
                    .reshape(128, NB * 768)).astype(NPBF)
        woc_c = won[:, c * 512:(c + 1) * 512]                    # [D, 512]
        woc_img = (woc_c.reshape(NB, 128, 512).transpose(1, 0, 2)
                   .reshape(128, NB * 512)).astype(NPBF)
        in_maps.append({
            "xtr": xtr,
            "wqkv": np.ascontiguousarray(wqkv_img),
            "woc": np.ascontiguousarray(woc_img),
            "cst": cstn,
            "snt": sntn,
            "m01": np.ascontiguousarray(m01),
            "ident": identity,
        })
    return in_maps


def kernel(x, wq, wk, wv, wo, cache_k, cache_v, freqs_cos, freqs_sin, mask,
           input_indexes):
    """Full-input / full-output entry point.  cache_k/cache_v/input_indexes are
    consumed by the reference semantics (zero cache fully overwritten at
    positions arange(S)), so keys/values equal the fresh projections."""
    in_maps = _prep_in_maps(x, wq, wk, wv, wo, freqs_cos, freqs_sin, mask)
    if "nc" not in _CACHE:
        _CACHE["nc"] = build_kernel()
    nc = _CACHE["nc"]
    res = run_bass_kernel_spmd(nc, in_maps, core_ids=list(range(N_CORES)))
    full = np.concatenate([res.results[c]["out"] for c in range(N_CORES)], axis=1)
    return full.reshape(1, S, D).astype(np.float32)


if __name__ == "__main__":
    rng = np.random.default_rng(0)
    ins = {
        "x": rng.standard_normal((1, S, D), dtype=np.float32),
        "wq": (rng.standard_normal((D, H * HD), dtype=np.float32) * 0.02),
        "wk": (rng.standard_normal((D, KVH * HD), dtype=np.float32) * 0.02),
        "wv": (rng.standard_normal((D, KVH * HD), dtype=np.float32) * 0.02),
        "wo": (rng.standard_normal((H * HD, D), dtype=np.float32) * 0.02),
        "cache_k": np.zeros((1, S, KVH, HD), np.float32),
        "cache_v": np.zeros((1, S, KVH, HD), np.float32),
        "freqs_cos": rng.random((S, HD // 2), dtype=np.float32),
        "freqs_sin": rng.random((S, HD // 2), dtype=np.float32),
        "mask": np.triu(np.full((S, S), -1e9, dtype=np.float32), k=1),
        "input_indexes": np.arange(S, dtype=np.int32),
    }
    o = kernel(**ins)
    print("kernel output", o.shape, o.dtype)


# revision 41
# speedup vs baseline: 3.5157x; 3.5157x over previous
"""Trainium2 8-core tensor-parallel attention kernel (Bass/Tile).

Problem: B=1, S=2048, D=4096, H=32 q-heads, KVH=8 kv-heads, HD=128, causal,
RoPE, GQA.  Sharding: tensor-parallel over heads — each of the 8 cores owns
4 q heads + 1 kv head (wq/wk/wv column shards), computes its heads' attention
output transposed [512, S], AllGathers those into [4096, S], and then computes
a 512-wide column shard of the final wo projection.  Host assembles the full
[1, S, 4096] output by concatenating the per-core column shards.

Compute dtype: bf16 operands / f32 accumulation (rel err ~0.8% vs f32 ref).
Layouts are host-prepared so every DMA is a contiguous partition-major image:
  xtr  [4, 128, 32*512]  x^T tiles: xtr[T][p, 512*b+f] = x[512*T+f, 128*b+p]
  wqkv [128, 32*768]     [wq_perm | wk_perm | wv] d-block-major stationary tiles
  woc  [128, 32*512]     wo column shard, d-block-major
  cst/snt [64, 2048]     RoPE tables transposed (f32)
  m01  [128, 4*512]      0/1 causal masks for the 4 diagonal block offsets
RoPE trick: wq/wk columns are permuted per head (even channels first) so the
rotation acts on partition halves [0:64]/[64:128] of the projected qT/kT.
Softmax is computed in the transposed score layout (scoresT[sk, sq]) without
max-subtraction (|scores| <~ 15 so exp stays in f32/bf16 range); column sums
via a ones-vector matmul, normalization via a rank-1 outer-product matmul.
"""

import math

import numpy as np
import ml_dtypes

import concourse.bass as bass
import concourse.bacc as bacc
import concourse.tile as tile
import concourse.mybir as mybir
from concourse.bass_utils import run_bass_kernel_spmd

BF16 = mybir.dt.bfloat16
F32 = mybir.dt.float32
NPBF = ml_dtypes.bfloat16

N_CORES = 8
S, D, H, KVH, HD = 2048, 4096, 32, 8, 128
QH = H // N_CORES          # 4 q heads per core
NB = D // 128              # 32 contraction blocks
ST = S // 512              # 4 s-tiles
SCALE = 1.0 / math.sqrt(HD)

_CACHE = {}


def build_kernel(n_cores=N_CORES, with_collective=True, reps=1,
                 skip_final=False, ag_splits=4):
    nc = bacc.Bacc("TRN2", target_bir_lowering=False, debug=False,
                   num_devices=n_cores)

    xtr = nc.declare_dram_parameter("xtr", [ST, 128, NB * 512], BF16, isOutput=False)
    wqkv = nc.declare_dram_parameter("wqkv", [128, NB * 768], BF16, isOutput=False)
    woc = nc.declare_dram_parameter("woc", [128, NB * 512], BF16, isOutput=False)
    cst = nc.declare_dram_parameter("cst", [64, S], F32, isOutput=False)
    snt = nc.declare_dram_parameter("snt", [64, S], F32, isOutput=False)
    m01 = nc.declare_dram_parameter("m01", [128, 4 * 512], BF16, isOutput=False)
    ident = nc.declare_dram_parameter("ident", [128, 128], BF16, isOutput=False)
    out = nc.declare_dram_parameter("out", [S, 512], F32, isOutput=True)

    with tile.TileContext(nc) as tc:
        with (
            tc.tile_pool(name="const", bufs=1) as constp,
            tc.tile_pool(name="dram", bufs=1, space="DRAM") as dram,
        ):
            # persistent SBUF images (chunked so the first matmuls can start
            # after ~1.5 MB instead of the full 6.3 MB; sync ring carries
            # wqkv, scalar ring carries xt + small constants)
            # wqkv as separate chunk tiles so the first matmuls only wait for
            # the first ~0.3 MB (Tile tracks deps per tile, not per slice)
            WQ_CHUNKS = ((0, 2), (2, 8), (8, 16), (16, 24), (24, 32))
            wqkv_sb = []
            for ci, (lo, hi) in enumerate(WQ_CHUNKS):
                t = constp.tile([128, 768 * (hi - lo)], BF16, name=f"wq_ch{ci}")
                nc.sync.dma_start(t[:], wqkv[:, 768 * lo: 768 * hi])
                wqkv_sb.append(t)
                if ci == 0:
                    cst_sb = constp.tile([64, S], F32)
                    nc.gpsimd.dma_start(cst_sb[:], cst[:])
                    snt_sb = constp.tile([64, S], F32)
                    nc.gpsimd.dma_start(snt_sb[:], snt[:])
                    m01_sb = constp.tile([128, 4 * 512], BF16)
                    nc.gpsimd.dma_start(m01_sb[:], m01[:])
                    ident_sb = constp.tile([128, 128], BF16)
                    nc.gpsimd.dma_start(ident_sb[:], ident[:])
            ones_sb = constp.tile([128, 128], BF16)
            nc.gpsimd.memset(ones_sb[:], 1.0)

            for rep in range(reps):
                nsp = max(1, ag_splits)
                cc_in = [dram.tile([QH * HD, S // nsp], BF16,
                                   name=f"cc_in{rep}_{half}")
                         for half in range(nsp)]
                cc_out = [dram.tile([N_CORES * QH * HD, S // nsp], BF16,
                                    addr_space="Shared",
                                    name=f"cc_out{rep}_{half}")
                          for half in range(nsp)]
                _emit_body(nc, tc, n_cores, with_collective,
                           xtr, woc, out, wqkv_sb, cst_sb, snt_sb,
                           m01_sb, ident_sb, ones_sb, cc_in, cc_out,
                           skip_final=skip_final, ag_splits=ag_splits)

    nc.compile()
    return nc


def _proj_epilogue(nc, work, psC, q_sb, kt_sb, v_sb, cst_sb, snt_sb, ident_sb,
                   scol, T, n, pp):
    if n < 5:
        # RoPE: rows 0:64 real, 64:128 imag (weights were permuted).
        # All four products first so the psum bank is released early.
        dst = q_sb[n] if n < QH else kt_sb
        c = cst_sb[:, scol]
        s_ = snt_sb[:, scol]
        t1 = work.tile([64, 512], F32, tag="rt1")
        t2 = work.tile([64, 512], F32, tag="rt2")
        t3 = work.tile([64, 512], F32, tag="rt3")
        t4 = work.tile([64, 512], F32, tag="rt4")
        nc.vector.tensor_mul(t1[:], pp[0:64, :], c)
        nc.vector.tensor_mul(t2[:], pp[64:128, :], s_)
        nc.vector.tensor_mul(t3[:], pp[0:64, :], s_)
        nc.vector.tensor_mul(t4[:], pp[64:128, :], c)
        nc.vector.tensor_sub(dst[0:64, scol], t1[:], t2[:])
        nc.vector.tensor_add(dst[64:128, scol], t3[:], t4[:])
    else:
        # V: cast to bf16, then PE-transpose 128x128 blocks into [sk, hd]
        vt = work.tile([128, 512], BF16, tag="vt")
        nc.scalar.copy(vt[:], pp[:])
        for j in range(4):
            pst = psC.tile([128, 128], BF16, tag="sc")
            nc.tensor.transpose(pst[:], vt[:, 128 * j: 128 * j + 128],
                                ident_sb[:])
            nc.vector.tensor_copy(
                v_sb[:, 128 * (4 * T + j): 128 * (4 * T + j) + 128],
                pst[:])


def _emit_body(nc, tc, n_cores, with_collective, xtr, woc, out, wqkv_sb,
               cst_sb, snt_sb, m01_sb, ident_sb, ones_sb, cc_in, cc_out,
               skip_final=False, ag_splits=2):
    with (
        tc.tile_pool(name="persist", bufs=1) as pers,
        tc.tile_pool(name="xs", bufs=2) as xpool,
        tc.tile_pool(name="work", bufs=2) as work,
        tc.tile_pool(name="exps", bufs=4) as epool,
        tc.tile_pool(name="psA", bufs=3, space="PSUM") as psA,
        tc.tile_pool(name="psB", bufs=2, space="PSUM") as psB,
        tc.tile_pool(name="psC", bufs=2, space="PSUM") as psC,
        tc.tile_pool(name="psD", bufs=1, space="PSUM") as psD,
    ):
        # per-core activations (bf16)
        q_sb = [pers.tile([128, S], BF16, name=f"q{h}_sb") for h in range(QH)]
        kt_sb = pers.tile([128, S], BF16)
        v_sb = pers.tile([128, S], BF16)          # [sk within blk, hd] per 128-col block

        WQ_CHUNKS = ((0, 2), (2, 8), (8, 16), (16, 24), (24, 32))
        XT_CHUNKS = ((0, 8), (8, 16), (16, 24), (24, 32))

        def wq_tile(d, n):
            for ci, (lo, hi) in enumerate(WQ_CHUNKS):
                if lo <= d < hi:
                    base = 768 * (d - lo) + 128 * n
                    return wqkv_sb[ci][:, base: base + 128]
            raise AssertionError

        for T in range(ST):
            xt_ch = []
            for ci, (lo, hi) in enumerate(XT_CHUNKS):
                t = xpool.tile([128, 512 * (hi - lo)], BF16, tag="xt", bufs=8)
                nc.scalar.dma_start(t[:], xtr[T][:, 512 * lo: 512 * hi])
                xt_ch.append(t)

            def xt_tile(d):
                ci = d // 8
                return xt_ch[ci][:, 512 * (d - 8 * ci): 512 * (d - 8 * ci) + 512]

            scol = slice(512 * T, 512 * T + 512)
            # ---- projections for this s-tile: k, v first, then 4 q heads.
            # n-pairs swept d-major so the first matmuls only need the first
            # weight/x chunks (startup), with 2 psum banks per pair.
            for npair in ((4, 5), (0, 1), (2, 3)):
                pps = {}
                for n in npair:
                    pps[n] = psA.tile([128, 512], F32, tag="proj",
                                      name=f"pp{n}")
                for d in range(NB):
                    for n in npair:
                        nc.tensor.matmul(
                            pps[n][:],
                            lhsT=wq_tile(d, n),
                            rhs=xt_tile(d),
                            start=(d == 0), stop=(d == NB - 1),
                        )
                for n in npair:
                    _proj_epilogue(nc, work, psC, q_sb, kt_sb, v_sb, cst_sb,
                                   snt_sb, ident_sb, scol, T, n, pps[n])

            # ---- attention for this s-tile, all 4 heads ----
            nsk = 4 * (T + 1)
            for h in range(QH):
                op = psB.tile([128, 512], F32, tag="outp")
                sm = psD.tile([1, 512], F32, tag="sums")
                ngrp = nsk // 4
                for b in range(nsk):
                    sc = psC.tile([128, 512], F32, tag="sc")
                    nc.tensor.matmul(
                        sc[:],
                        lhsT=kt_sb[:, 128 * b: 128 * b + 128],
                        rhs=q_sb[h][:, scol],
                        start=True, stop=True,
                    )
                    e = epool.tile([128, 512], BF16, tag="e")
                    nc.scalar.activation(e[:], sc[:],
                                         mybir.ActivationFunctionType.Exp,
                                         scale=SCALE)
                    r = b - 4 * T
                    if r >= 0:
                        nc.vector.tensor_mul(
                            e[:], e[:], m01_sb[:, 512 * r: 512 * r + 512])
                    # accumulate exp tiles in groups of 4 on DVE; one
                    # ones-matmul per group instead of per block
                    gpos = b % 4
                    if gpos == 0:
                        ea = epool.tile([128, 512], BF16, tag="ea", bufs=2)
                        nc.vector.tensor_copy(ea[:], e[:])
                    else:
                        nc.vector.tensor_add(ea[:], ea[:], e[:])
                    if gpos == 3:
                        nc.tensor.matmul(sm[0:1, :], lhsT=ones_sb[:, 0:1],
                                         rhs=ea[:],
                                         start=(b == 3), stop=(b == nsk - 1))
                    nc.tensor.matmul(op[:],
                                     lhsT=v_sb[:, 128 * b: 128 * b + 128],
                                     rhs=e[:],
                                     start=(b == 0), stop=(b == nsk - 1))
                # normalize: og = op * (1/sums) broadcast across partitions
                r32 = work.tile([1, 512], F32, tag="r32")
                nc.vector.reciprocal(r32[:], sm[0:1, :])
                rbf = work.tile([1, 512], BF16, tag="rbf")
                nc.vector.tensor_copy(rbf[:], r32[:])
                bcs = work.tile([128, 512], BF16, tag="bcs")
                nc.gpsimd.partition_broadcast(bcs[:], rbf[:])
                og = work.tile([128, 512], BF16, tag="og", bufs=3)
                nc.vector.tensor_mul(og[:], op[:], bcs[:])
                tps = ST // max(1, ag_splits)   # s-tiles per split
                nc.sync.dma_start(
                    cc_in[T // tps][128 * h: 128 * h + 128,
                                    512 * (T % tps): 512 * (T % tps) + 512],
                    og[:])

            # issue the AllGather for each completed s-chunk so it overlaps
            # the remaining attention / final compute
            tps = ST // max(1, ag_splits)
            halves = [T // tps] if (T + 1) % tps == 0 else []
            for half in halves:
                if with_collective:
                    nc.gpsimd.collective_compute(
                        "AllGather",
                        mybir.AluOpType.bypass,
                        replica_groups=[list(range(n_cores))],
                        ins=[cc_in[half].opt()],
                        outs=[cc_out[half].opt()],
                    )
                else:
                    # timing-only stand-in: fill own shard slot locally
                    nc.sync.dma_start(cc_out[half][0: QH * HD, :],
                                      cc_in[half][:])

    if skip_final:
        with tc.tile_pool(name="tailp", bufs=1) as tp:
            t = tp.tile([128, 512], F32)
            nc.gpsimd.dma_start(t[:], cc_out[-1][0:128, 0:512])
            nc.sync.dma_start(out[0:128, :], t[:])
        return

    # ---- final projection: out[:, core cols] = outT_all^T @ woc ----
    # woc/gather tiles are split along the contraction dim (c-halves of 16
    # blocks) so the first matmuls start after ~2 MB of DMA, not 8.
    with (
        tc.tile_pool(name="fing", bufs=3) as gpool,
        tc.tile_pool(name="finw", bufs=1) as wpool,
        tc.tile_pool(name="fino", bufs=3) as opool,
        tc.tile_pool(name="psF", bufs=4, space="PSUM") as psF,
    ):
        woc_h = []
        for ch in range(2):
            wt = wpool.tile([128, 16 * 512], BF16, name=f"woc_h{ch}")
            nc.sync.dma_start(wt[:], woc[:, 16 * 512 * ch: 16 * 512 * (ch + 1)])
            woc_h.append(wt)
        nsp = max(1, ag_splits)
        qps = 4 // nsp          # quarters per split
        cc_views = [cc_out[half][:, :].rearrange("(b p) s -> p b s", p=128)
                    for half in range(nsp)]
        for qt in range(4):
            view = cc_views[qt // qps]
            colq = 512 * (qt % qps)
            gsb_h = []
            for ch in range(2):
                g = gpool.tile([128, 16, 512], BF16, tag="g")
                nc.sync.dma_start(
                    g[:], view[:, 16 * ch: 16 * (ch + 1), colq: colq + 512])
                gsb_h.append(g)
            for j in range(4):
                pf = psF.tile([128, 512], F32, tag="fin")
                for b in range(NB):
                    ch, bi = b // 16, b % 16
                    nc.tensor.matmul(
                        pf[:],
                        lhsT=gsb_h[ch][:, bi, 128 * j: 128 * j + 128],
                        rhs=woc_h[ch][:, 512 * bi: 512 * bi + 512],
                        start=(b == 0), stop=(b == NB - 1),
                    )
                osb = opool.tile([128, 512], F32, tag="o")
                nc.vector.tensor_copy(osb[:], pf[:])
                nc.sync.dma_start(out[512 * qt + 128 * j: 512 * qt + 128 * j + 128, :],
                                  osb[:])


# ---------------------------------------------------------------------------
# host-side preparation


def _prep_in_maps(x, wq, wk, wv, wo, freqs_cos, freqs_sin, mask):
    x2 = np.asarray(x, np.float32).reshape(S, D)
    perm = np.concatenate([np.arange(0, HD, 2), np.arange(1, HD, 2)])

    xT = np.ascontiguousarray(x2.T).astype(NPBF)                 # [D, S]
    # xtr[T][p, 512*b+f] = xT[128*b+p, 512*T+f]
    xtr = (xT.reshape(NB, 128, ST, 512).transpose(2, 1, 0, 3)
           .reshape(ST, 128, NB * 512))
    xtr = np.ascontiguousarray(xtr)

    cstn = np.ascontiguousarray(np.asarray(freqs_cos, np.float32).T)  # [64, S]
    sntn = np.ascontiguousarray(np.asarray(freqs_sin, np.float32).T)

    mnp = np.asarray(mask, np.float32)
    m01 = np.zeros((128, 4 * 512), np.float32)
    for r in range(4):
        m01[:, 512 * r: 512 * r + 512] = (mnp[0:512, 128 * r: 128 * r + 128].T == 0.0)
    m01 = m01.astype(NPBF)

    identity = np.eye(128, dtype=NPBF)

    wqn = np.asarray(wq, np.float32)
    wkn = np.asarray(wk, np.float32)
    wvn = np.asarray(wv, np.float32)
    won = np.asarray(wo, np.float32)

    in_maps = []
    for c in range(N_CORES):
        wq_c = wqn[:, c * QH * HD:(c + 1) * QH * HD].reshape(D, QH, HD)[:, :, perm]
        wq_c = wq_c.reshape(D, QH * HD)
        wk_c = wkn[:, c * HD:(c + 1) * HD][:, perm]
        wv_c = wvn[:, c * HD:(c + 1) * HD]
        wqkv_c = np.concatenate([wq_c, wk_c, wv_c], axis=1)      # [D, 768]
        wqkv_img = (wqkv_c.reshape(NB, 128, 768).transpose(1, 0, 2)
                    .reshape(128, NB * 768)).astype(NPBF)
        woc_c = won[:, c * 512:(c + 1) * 512]                    # [D, 512]
        woc_img = (woc_c.reshape(NB, 128, 512).transpose(1, 0, 2)
                   .reshape(128, NB * 512)).astype(NPBF)
        in_maps.append({
            "xtr": xtr,
            "wqkv": np.ascontiguousarray(wqkv_img),
            "woc": np.ascontiguousarray(woc_img),
            "cst": cstn,
            "snt": sntn,
            "m01": np.ascontiguousarray(m01),
            "ident": identity,
        })
    return in_maps


def kernel(x, wq, wk, wv, wo, cache_k, cache_v, freqs_cos, freqs_sin, mask,
           input_indexes):
    """Full-input / full-output entry point.  cache_k/cache_v/input_indexes are
    consumed by the reference semantics (zero cache fully overwritten at
    positions arange(S)), so keys/values equal the fresh projections."""
    in_maps = _prep_in_maps(x, wq, wk, wv, wo, freqs_cos, freqs_sin, mask)
    if "nc" not in _CACHE:
        _CACHE["nc"] = build_kernel()
    nc = _CACHE["nc"]
    res = run_bass_kernel_spmd(nc, in_maps, core_ids=list(range(N_CORES)))
    full = np.concatenate([res.results[c]["out"] for c in range(N_CORES)], axis=1)
    return full.reshape(1, S, D).astype(np.float32)


if __name__ == "__main__":
    rng = np.random.default_rng(0)
    ins = {
        "x": rng.standard_normal((1, S, D), dtype=np.float32),
        "wq": (rng.standard_normal((D, H * HD), dtype=np.float32) * 0.02),
        "wk": (rng.standard_normal((D, KVH * HD), dtype=np.float32) * 0.02),
        "wv": (rng.standard_normal((D, KVH * HD), dtype=np.float32) * 0.02),
        "wo": (rng.standard_normal((H * HD, D), dtype=np.float32) * 0.02),
        "cache_k": np.zeros((1, S, KVH, HD), np.float32),
        "cache_v": np.zeros((1, S, KVH, HD), np.float32),
        "freqs_cos": rng.random((S, HD // 2), dtype=np.float32),
        "freqs_sin": rng.random((S, HD // 2), dtype=np.float32),
        "mask": np.triu(np.full((S, S), -1e9, dtype=np.float32), k=1),
        "input_indexes": np.arange(S, dtype=np.int32),
    }
    o = kernel(**ins)
    print("kernel output", o.shape, o.dtype)
